# revision 2
# baseline (speedup 1.0000x reference)
"""Trainium2 Bass kernel for ChannelTransformerBlock — v3 (skewed pipeline).

On top of v2 (fp8 DoubleRow attention + fc2, bf16 fc1, merged transpose
evacuations, fc2 operand swap):
  - The two batches per core are software-pipelined with a half-layer skew:
    batch 1's matmul-dense phases (qkv, MLP) are emitted interleaved with
    batch 0's PE-sparse phases (LN, softmax, factors) so the PE never idles
    long enough for HAM to re-throttle the clock to 1.2 GHz.
  - Residual stream kept in bf16 (x cast once on gpsimd), halving DVE
    bytes for LN stats/apply and residual adds.
  - LN scalar chains batched 4 tiles per sqrt/reciprocal instruction.
  - Softmax processes head PAIRS on 96 partitions (half the instructions);
    the exp is evaluated only on the two diagonal 48x48 blocks.
  - Sum-of-squares accumulates into a single PSUM bank ([2,C] via two
    one-column selector matrices); raw sums round-trip through DRAM and
    the rsqrt math happens on the reloaded [96,x] tiles.
  - zs is never materialized: each 128-column attnout block feeds exactly
    one proj transpose, so psz evacuates into a tiny ring and proj runs
    fused with Z as blocks become ready.
"""

import sys

import numpy as np

try:
    import concourse.bass as bass  # noqa: F401
except ImportError:  # pragma: no cover
    for _p in ("/opt/trn_rl_repo", "/root/.axon_site/_ro/trn_rl_repo"):
        if _p not in sys.path:
            sys.path.insert(0, _p)

import os
import ml_dtypes
from contextlib import ExitStack

import concourse.bass as bass
import concourse.mybir as mybir
from concourse import bacc
import concourse.tile as tile
from concourse.bass import ts
from concourse.bass_utils import run_bass_kernel_spmd
from concourse.masks import make_identity

F32 = mybir.dt.float32
BF16 = mybir.dt.bfloat16
F8 = mybir.dt.float8e4
AF = mybir.ActivationFunctionType
ALU = mybir.AluOpType
DRM = mybir.MatmulPerfMode.DoubleRow

SIM_GELU_IDENTITY = bool(os.environ.get("SIM_GELU_IDENTITY"))

# Problem constants (hardcoded per task spec).
B, N, C = 16, 4096, 384
H, HD = 8, 48
HID = 4 * C
SCALE = HD ** -0.5
EPS = 1e-5
NCORES = 8
BL = B // NCORES          # batches per core
P = 128
NT = N // P               # token tiles per batch (32)
NTH = NT // 2
NTQ = NT // 4             # token tiles per quarter (8)
CK = C // P               # channel chunks (3)
HK = HID // P             # hidden chunks (12)
NB = 512
NNB = N // NB             # 8
NBLK = 2048
NBLKS = N // NBLK         # 2
GPB = NBLK // P           # 16 g-tiles per MLP block
WS = 64.0
WSV = 512.0
DQ = 1.0 / WS
DQZ = 1.0 / WSV

# proj readiness: block (rho, mu) done after nbi = mu // 4; g ready when all
# of q3 = 3g+kj (kj=0..2) with rho = q3 // NT, mu = q3 % NT are done.
_READY = {nbi: [] for nbi in range(NNB)}
for _g in range(NT):
    _need = max((3 * _g + _kj) % NT // 4 for _kj in range(CK))
    _READY[_need].append(_g)


def build_program(apply_pjb: bool, apply_qkb: bool = False,
                  apply_f2b: bool = False, nbatch: int = BL):
    nc = bacc.Bacc()

    x_d = nc.declare_dram_parameter("x", [nbatch, N, C], F32, isOutput=False)
    wqk_d = nc.declare_dram_parameter("wqk", [C, 2 * C], F8, isOutput=False)
    qkb_d = nc.declare_dram_parameter("qkb", [2 * C], F32, isOutput=False)
    wvt_d = nc.declare_dram_parameter("wvt", [C, C], F8, isOutput=False)
    wpr_d = nc.declare_dram_parameter("wpr", [C, C], F8, isOutput=False)
    pjb_d = nc.declare_dram_parameter("pjb", [C], F32, isOutput=False)
    wf1_d = nc.declare_dram_parameter("wf1", [C, HID], BF16, isOutput=False)
    f1b_d = nc.declare_dram_parameter("f1b", [HID], F32, isOutput=False)
    wf2_d = nc.declare_dram_parameter("wf2", [HID, C], F8, isOutput=False)
    f2b_d = nc.declare_dram_parameter("f2b", [C], F32, isOutput=False)
    y_d = nc.declare_dram_parameter("y", [nbatch, N, C], F32, isOutput=True)

    with tile.TileContext(nc) as tc, ExitStack() as ctx:
        w = ctx.enter_context(tc.tile_pool(name="w", bufs=1))
        xres = ctx.enter_context(tc.tile_pool(name="xres", bufs=2))
        htokp = ctx.enter_context(tc.tile_pool(name="htokp", bufs=4))
        hTp = ctx.enter_context(tc.tile_pool(name="hTp", bufs=2))
        bigp = ctx.enter_context(tc.tile_pool(name="bigp", bufs=2))
        qtp = ctx.enter_context(tc.tile_pool(name="qtp", bufs=2))
        scrp = ctx.enter_context(tc.tile_pool(name="scrp", bufs=1))
        pjp = ctx.enter_context(tc.tile_pool(name="pjp", bufs=2))
        asm = ctx.enter_context(tc.tile_pool(name="asm", bufs=1))
        tiny = ctx.enter_context(tc.tile_pool(name="tiny", bufs=3))
        outp = ctx.enter_context(tc.tile_pool(name="outp", bufs=2))
        ps_big = ctx.enter_context(tc.tile_pool(name="ps_big", bufs=4, space="PSUM"))
        ps_sq = ctx.enter_context(tc.tile_pool(name="ps_sq", bufs=1, space="PSUM"))
        ps_sc = ctx.enter_context(tc.tile_pool(name="ps_sc", bufs=1, space="PSUM"))
        ps_t = ctx.enter_context(tc.tile_pool(name="ps_t", bufs=2, space="PSUM"))
        dram = ctx.enter_context(tc.tile_pool(name="dram", bufs=2, space="DRAM"))

        # ---- constants / weights ----
        wqk_sb = w.tile([P, CK, 2 * C], F8)
        nc.gpsimd.dma_start(wqk_sb[:], wqk_d.rearrange("(k p) m -> p k m", p=P))
        qkb_sb = None
        if apply_qkb:
            qkb_sb = w.tile([P, 2 * C], F32)
            _a = qkb_d[:]
            nc.gpsimd.dma_start(qkb_sb[:], bass.AP(
                tensor=_a.tensor, offset=_a.offset, ap=[[0, P], [1, 2 * C]]))
        # head-pair interleaved wvt: partition ee = (h%2)*48 + d, pr = h//2
        wvtp_sb = w.tile([2 * HD, H // 2, C], F8)
        nc.gpsimd.dma_start(wvtp_sb[:],
                            wvt_d.rearrange("(pr ee) c -> ee pr c", ee=2 * HD))
        wpr_sb = w.tile([P, CK, C], F8)
        nc.gpsimd.dma_start(wpr_sb[:], wpr_d.rearrange("(k p) m -> p k m", p=P))
        wf1_sb = w.tile([P, CK, HID], BF16)
        nc.gpsimd.dma_start(wf1_sb[:], wf1_d.rearrange("(k p) m -> p k m", p=P))
        f1b_sb = w.tile([P, HK], F32)
        nc.gpsimd.dma_start(f1b_sb[:], f1b_d.rearrange("(j p) -> p j", p=P))
        wf2_sb = w.tile([P, HK, C], F8)
        nc.gpsimd.dma_start(wf2_sb[:], wf2_d.rearrange("(k p) m -> p k m", p=P))
        pjb_sb = None
        if apply_pjb:
            pjb_sb = w.tile([P, C], F32)
            _a = pjb_d[:]
            nc.gpsimd.dma_start(pjb_sb[:], bass.AP(
                tensor=_a.tensor, offset=_a.offset, ap=[[0, P], [1, C]]))
        f2bb_sb = None
        if apply_f2b:
            f2bb_sb = w.tile([P, C], F32)
            _a = f2b_d[:]
            nc.gpsimd.dma_start(f2bb_sb[:], bass.AP(
                tensor=_a.tensor, offset=_a.offset, ap=[[0, P], [1, C]]))

        ident16 = w.tile([P, P], BF16)
        make_identity(nc, ident16[:])
        # selector columns: oc[:, 0, :] = [1, 0], oc[:, 1, :] = [0, 1]
        ocol = w.tile([P, 2, 2], BF16)
        nc.vector.memset(ocol[:], 0.0)
        nc.vector.memset(ocol[:, 0, 0:1], 1.0)
        nc.vector.memset(ocol[:, 1, 1:2], 1.0)
        eps_sb = w.tile([P, 1], F32)
        nc.vector.memset(eps_sb[:], EPS)

        rr = {"i": 0}

        def cp_rr(dst, src):
            i = rr["i"]
            rr["i"] += 1
            if i % 2 == 0:
                nc.vector.tensor_copy(dst, src)
            else:
                nc.scalar.copy(dst, src)

        st8 = {}  # per-batch emitter state

        # ================= phase generators =================
        def ph_load(b):
            xb = xres.tile([P, NT, C], BF16, tag="xb", name=f"xb{b}")
            st8[b] = {"xb": xb}
            for qi in range(8):
                xf = xres.tile([P, NT // 8, C], F32, tag="xf",
                               name=f"xf{b}_{qi}", bufs=1)
                nc.gpsimd.dma_start(
                    xf[:],
                    x_d[b].rearrange("(a s) c -> a s c",
                                     s=NT)[:, ts(qi, NT // 8), :])
                nc.gpsimd.tensor_copy(xb[:, ts(qi, NT // 8), :], xf[:])
                yield

        def x_ap(b, g):
            return st8[b]["xb"][:, g, :]

        def ln_group4(b, g0, dst4):
            # dst4(i) -> destination ht AP for tile g0+i; batched sqrt/recip
            mv4 = tiny.tile([P, 4, 2], F32, tag="mv4", name="mv4")
            for i in range(4):
                stt = tiny.tile([P, 6], F32, tag="bnstats", name="stt")
                nc.vector.bn_stats(stt[:], x_ap(b, g0 + i))
                nc.vector.bn_aggr(mv4[:, i, :], stt[:])
            rs4 = tiny.tile([P, 4], F32, tag="rs4", name="rs4")
            nc.scalar.activation(rs4[:], mv4[:, :, 1], AF.Sqrt,
                                 bias=eps_sb[:, 0:1], scale=1.0)
            nc.vector.reciprocal(rs4[:], rs4[:])
            for i in range(4):
                nc.vector.tensor_scalar(dst4(i), x_ap(b, g0 + i),
                                        scalar1=mv4[:, i, 0:1],
                                        scalar2=rs4[:, i:i + 1],
                                        op0=ALU.subtract, op1=ALU.mult)

        def ph_ln(b, which):
            # which = 1: h1T fp8, true-token column order (strided dst)
            # which = 2: h2T bf16, grouped column order (contiguous dst)
            if which == 1:
                hT = hTp.tile([P, CK, N], F8, tag="h1T", name=f"h1T{b}")
                hTm = hT.rearrange("p j (a s) -> p s j a", s=NT)
                st8[b]["h1T"] = hT
            else:
                hT = hTp.tile([P, CK, N], F8, tag="h2T", name=f"h2T{b}")
                hTm = hT.rearrange("p j (s a) -> p s j a", a=P)
                st8[b]["h2T"] = hT
            for g4i in range(NT // 4):
                g0 = g4i * 4
                hts = [htokp.tile([P, C], BF16, tag="htok", name=f"ht{i}")
                       for i in range(4)]
                ln_group4(b, g0, lambda i: hts[i][:])
                for sub in range(2):
                    pt = ps_t.tile([P, 6, P], BF16, tag="pt", name="pt")
                    for gi in range(2):
                        ht = hts[sub * 2 + gi]
                        for j in range(CK):
                            nc.tensor.transpose(pt[:, gi * CK + j, :],
                                                ht[:, ts(j, P)], ident16[:])
                    cp_rr(hTm[:, g0 + sub * 2: g0 + sub * 2 + 2, :, :], pt[:])
                yield

        def ph_qkv(b):
            # k-side of qt is stored pair-gapped: head h=2*pr+two at columns
            # C + pr*128 + two*64 + d, so a single 112-wide lhsT slice yields
            # score blocks at legal partition bases 0 and 64.
            h1T = st8[b]["h1T"]
            psqk = ps_sq.tile([2, C], F32, tag="sq", name=f"psqk{b}")
            pscore = ps_sc.tile([112, H // 2, 2 * HD], F32, tag="score",
                                name=f"pscore{b}")
            st8[b]["pscore"] = pscore
            for t in range(NT):
                psa = ps_big.tile([P, C], F32, tag="big", name="psa")
                psb = ps_big.tile([P, C], F32, tag="big", name="psb")
                lhs2 = h1T[:, 0:2, ts(t, P)]
                lhs1 = h1T[:, 2, ts(t, P)]
                nc.tensor.matmul(psa[:], lhs2, wqk_sb[:, 0:2, 0:C],
                                 start=True, stop=False, perf_mode=DRM)
                nc.tensor.matmul(psb[:], lhs2, wqk_sb[:, 0:2, C:2 * C],
                                 start=True, stop=False, perf_mode=DRM)
                nc.tensor.matmul(psa[:], lhs1, wqk_sb[:, 2, 0:C],
                                 start=False, stop=True)
                nc.tensor.matmul(psb[:], lhs1, wqk_sb[:, 2, C:2 * C],
                                 start=False, stop=True)
                qt = qtp.tile([P, 2 * C], BF16, tag="qt", name="qt")
                if apply_qkb:
                    nc.vector.scalar_tensor_tensor(
                        qt[:, 0:C], psa[:], DQ, qkb_sb[:, 0:C],
                        op0=ALU.mult, op1=ALU.add)
                    nc.vector.scalar_tensor_tensor(
                        qt[:, C:2 * C], psb[:], DQ, qkb_sb[:, C:2 * C],
                        op0=ALU.mult, op1=ALU.add)
                else:
                    nc.vector.tensor_scalar_mul(qt[:, 0:C], psa[:], DQ)
                    nc.scalar.activation(qt[:, C:2 * C], psb[:],
                                         AF.Identity, scale=DQ)
                scr = scrp.tile([P, 2 * C], BF16, tag="scr", name="scr")
                nc.vector.tensor_mul(scr[:], qt[:], qt[:])
                nc.tensor.matmul(psqk[:], ocol[:, 0, :], scr[:, 0:C],
                                 start=(t == 0), stop=False)
                nc.tensor.matmul(psqk[:], ocol[:, 1, :], scr[:, C:2 * C],
                                 start=False, stop=(t == NT - 1))
                for pr in range(H // 2):
                    # head pair at partition bases 0 / 64 (32-aligned), with
                    # the pair's q columns as a shared 96-wide moving operand
                    nc.tensor.matmul(
                        pscore[0:HD, pr, :],
                        qt[:, C + (2 * pr) * HD:C + (2 * pr + 1) * HD],
                        qt[:, ts(pr, 96)],
                        start=(t == 0 and pr == 0),
                        stop=(t == NT - 1 and pr == H // 2 - 1),
                        skip_group_check=True)
                    nc.tensor.matmul(
                        pscore[64:64 + HD, pr, :],
                        qt[:, C + (2 * pr + 1) * HD:C + (2 * pr + 2) * HD],
                        qt[:, ts(pr, 96)],
                        start=(t == 0 and pr == 0),
                        stop=(t == NT - 1 and pr == H // 2 - 1),
                        skip_group_check=True)
                yield
            s_dram = dram.tile([2 * C], F32, tag="s_dram", name=f"s_dram{b}")
            st8[b]["s_dram"] = s_dram
            sq_sb = asm.tile([2, C], F32, tag="sq_sb", name=f"sq_sb{b}")
            nc.vector.tensor_copy(sq_sb[:], psqk[:])
            nc.gpsimd.dma_start(s_dram.rearrange("(r c) -> r c", r=2),
                                sq_sb[:])

        def ph_factors(b):
            s_dram = st8[b]["s_dram"]
            # skp rows pair-gapped like qt's k columns (heads at 0:48, 64:112)
            skp = asm.tile([P, H // 2], F32, tag="skp", name=f"skp{b}")
            nc.vector.memset(skp[:], 1.0)
            skv = s_dram[C:2 * C].rearrange("(pr two d) -> two d pr",
                                            two=2, d=HD)
            nc.gpsimd.dma_start(skp[0:HD, :], skv[0])
            nc.gpsimd.dma_start(skp[64:64 + HD, :], skv[1])
            sqbp = asm.tile([P, C], F32, tag="sqbp", name=f"sqbp{b}")
            nc.gpsimd.dma_start(sqbp[:], bass.AP(
                tensor=s_dram.tensor, offset=s_dram.offset,
                ap=[[0, P], [1, C]]))
            for sl in (slice(0, HD), slice(64, 64 + HD)):
                nc.vector.tensor_scalar_max(skp[sl, :], skp[sl, :], 1e-24)
            nc.scalar.sqrt(skp[0:112, :], skp[0:112, :])
            nc.vector.reciprocal(skp[0:112, :], skp[0:112, :])
            nc.vector.tensor_scalar_mul(skp[0:112, :], skp[0:112, :], SCALE)
            yield
            nc.vector.tensor_scalar_max(sqbp[:], sqbp[:], 1e-24)
            nc.scalar.sqrt(sqbp[:], sqbp[:])
            nc.vector.reciprocal(sqbp[:], sqbp[:])
            st8[b]["skp"] = skp
            st8[b]["sqbp"] = sqbp
            yield

        def ph_softmax(b):
            pscore = st8[b]["pscore"]
            skp, sqbp = st8[b]["skp"], st8[b]["sqbp"]
            rsallp = asm.tile([112, H // 2], F32, tag="rsallp",
                              name=f"rsallp{b}")
            nc.vector.memset(rsallp[:], 1.0)
            mft = asm.tile([P, CK, 3, 16, 8], F8, tag="mft", name=f"mft{b}")
            st8[b]["mft"] = mft
            # release pscore early: all 4 t1 reads first
            t1s = []
            for pr in range(H // 2):
                t1p = tiny.tile([112, 2 * HD], F32, tag="t1", name="t1p",
                                bufs=4)
                for sl in (slice(0, HD), slice(64, 112)):
                    nc.vector.scalar_tensor_tensor(
                        t1p[sl, :], pscore[sl, pr, :], skp[sl, pr:pr + 1],
                        sqbp[sl, pr * 96:(pr + 1) * 96],
                        op0=ALU.mult, op1=ALU.mult)
                t1s.append(t1p)
            for pr in range(H // 2):
                t1p = t1s[pr]
                e1p = tiny.tile([112, 2 * HD], BF16, tag="e1", name="e1p",
                                bufs=2)
                nc.vector.memset(e1p[:], 0.0)
                rsum = tiny.tile([112, 1], F32, tag="rsum", name="rsum")
                nc.scalar.activation(e1p[0:HD, 0:HD], t1p[0:HD, 0:HD],
                                     AF.Exp, accum_out=rsum[0:HD, :])
                nc.scalar.activation(e1p[64:112, HD:2 * HD],
                                     t1p[64:112, HD:2 * HD],
                                     AF.Exp, accum_out=rsum[64:112, :])
                nc.vector.reciprocal(rsallp[0:HD, pr:pr + 1], rsum[0:HD, :])
                nc.vector.reciprocal(rsallp[64:112, pr:pr + 1],
                                     rsum[64:112, :])
                ptw = ps_t.tile([2 * HD, 112], BF16, tag="pt", name="ptw")
                nc.tensor.transpose(ptw[:], e1p[:], ident16[:112, :112])
                atp = tiny.tile([2 * HD, 112], BF16, tag="at", name="atp",
                                bufs=2)
                nc.vector.tensor_copy(atp[:], ptw[:])
                psm = ps_t.tile([112, C], F32, tag="pt", name="psm")
                nc.tensor.matmul(psm[:], atp[:], wvtp_sb[:, pr, :],
                                 start=True, stop=True)
                mp = tiny.tile([112, C], BF16, tag="mp", name="mp",
                               bufs=2)
                nc.vector.tensor_scalar_mul(mp[:], psm[:],
                                            rsallp[:, pr:pr + 1])
                for j in range(CK):
                    ptm = ps_t.tile([P, 112], BF16, tag="pt", name="ptm")
                    nc.tensor.transpose(ptm[:], mp[:, ts(j, P)],
                                        ident16[:112, :112])
                    for sdx in range(2):
                        h = 2 * pr + sdx
                        ptm3 = ptm[:, 64 * sdx:64 * sdx + HD].rearrange(
                            "p (e q) -> p e q", q=3)
                        for dlt in range(3):
                            r0 = 8 * dlt + h
                            rho, az0 = (r0 % 3), (r0 // 3)
                            dst = mft[:, j, rho, :, az0]
                            if (h + j + dlt) % 2 == 0:
                                nc.scalar.copy(dst, ptm3[:, :, dlt])
                            else:
                                nc.vector.tensor_copy(dst, ptm3[:, :, dlt])
                yield

        def ph_zproj(b):
            h1T = st8[b]["h1T"]
            mftf = st8[b]["mft"].rearrange("p j r e s -> p j (r e s)")
            xb = st8[b]["xb"]
            zs = bigp.tile([P, CK, N], BF16, tag="dd", name=f"zs{b}")
            for nbi in range(NNB):
                for m in range(CK):
                    psz = ps_big.tile([P, NB], F32, tag="big", name="psz")
                    nc.tensor.matmul(psz[:], mftf[:, 0:2, ts(m, P)],
                                     h1T[:, 0:2, ts(nbi, NB)],
                                     start=True, stop=False, perf_mode=DRM)
                    nc.tensor.matmul(psz[:], mftf[:, 2, ts(m, P)],
                                     h1T[:, 2, ts(nbi, NB)],
                                     start=False, stop=True)
                    if m % 2 == 0:
                        nc.vector.tensor_scalar_mul(zs[:, m, ts(nbi, NB)],
                                                    psz[:], DQZ)
                    else:
                        nc.scalar.activation(zs[:, m, ts(nbi, NB)], psz[:],
                                             AF.Identity, scale=DQZ)
                yield
                for g in _READY[nbi]:
                    ptp = ps_t.tile([P, CK, P], BF16, tag="pt", name="ptp")
                    for kj in range(CK):
                        q3 = 3 * g + kj
                        rho, mu = q3 // NT, q3 % NT
                        nc.tensor.transpose(ptp[:, kj, :],
                                            zs[:, rho, ts(mu, P)], ident16[:])
                    pj = pjp.tile([P, CK, P], F8, tag="pj", name="pj")
                    cp_rr(pj[:], ptp[:])
                    pspr = ps_big.tile([P, C], F32, tag="big", name="pspr")
                    nc.tensor.matmul(pspr[:], pj[:, 0:2, :], wpr_sb[:, 0:2, :],
                                     start=True, stop=False, perf_mode=DRM)
                    nc.tensor.matmul(pspr[:], pj[:, 2, :], wpr_sb[:, 2, :],
                                     start=False, stop=True)
                    nc.vector.scalar_tensor_tensor(
                        xb[:, g, :], pspr[:], DQ, xb[:, g, :],
                        op0=ALU.mult, op1=ALU.add)
                    if apply_pjb:
                        nc.vector.tensor_add(xb[:, g, :], xb[:, g, :],
                                             pjb_sb[:])
                yield

        def ph_fc1(b, blk):
            h2T = st8[b]["h2T"]
            n0 = blk * NBLK
            g4t = bigp.tile([P, HK, NBLK], F8, tag="dd", name=f"g4t{b}_{blk}")
            st8[b][f"g4t{blk}"] = g4t
            gelu_af = AF.Identity if SIM_GELU_IDENTITY else AF.Gelu
            for m in range(HK):
                for half in range(2):
                    psums = [ps_big.tile([P, NB], F32, tag="big",
                                         name=f"pf{i}") for i in range(2)]
                    for kj in range(CK):
                        for i in range(2):
                            ii = half * 2 + i
                            nc.tensor.matmul(
                                psums[i][:], wf1_sb[:, kj, ts(m, P)],
                                h2T[:, kj, n0 + ii * NB:n0 + (ii + 1) * NB],
                                start=(kj == 0), stop=(kj == CK - 1))
                    for i in range(2):
                        ii = half * 2 + i
                        nc.scalar.activation(g4t[:, m, ts(ii, NB)],
                                             psums[i][:], gelu_af,
                                             bias=f1b_sb[:, m:m + 1],
                                             scale=1.0)
                yield

        def ph_fc2(b, blk):
            xb = st8[b]["xb"]
            g4t = st8[b][f"g4t{blk}"]
            yg = y_d[b].rearrange("(a s) c -> a s c", s=NT)
            for gg in range(GPB):
                g = blk * GPB + gg
                psf2 = ps_big.tile([P, C], F32, tag="big", name="pg0")
                for kjp in range(0, HK, 2):
                    nc.tensor.matmul(
                        psf2[:], g4t[:, kjp:kjp + 2, ts(gg, P)],
                        wf2_sb[:, kjp:kjp + 2, :],
                        start=(kjp == 0), stop=(kjp == HK - 2),
                        perf_mode=DRM)
                yt = outp.tile([P, C], F32, tag="yt", name="yt")
                nc.vector.scalar_tensor_tensor(
                    yt[:], psf2[:], DQ, xb[:, g, :], op0=ALU.mult, op1=ALU.add)
                if apply_f2b:
                    nc.vector.tensor_add(yt[:], yt[:], f2bb_sb[:])
                nc.gpsimd.dma_start(yg[:, g, :], yt[:])
                yield

        # ================= schedule =================
        def chain(*gens):
            for g in gens:
                yield from g

        def drive(g):
            for _ in g:
                pass

        def co(ga, gb, na=1, nb=1):
            a_done = b_done = False
            while not (a_done and b_done):
                if not a_done:
                    for _ in range(na):
                        try:
                            next(ga)
                        except StopIteration:
                            a_done = True
                            break
                if not b_done:
                    for _ in range(nb):
                        try:
                            next(gb)
                        except StopIteration:
                            b_done = True
                            break

        def attn_tail(b):
            return chain(ph_factors(b), ph_softmax(b), ph_zproj(b),
                         ph_ln(b, 2))

        def mlp(b):
            return chain(ph_fc1(b, 0), ph_fc2(b, 0), ph_fc1(b, 1),
                         ph_fc2(b, 1))

        if nbatch == 2:
            drive(ph_load(0))
            drive(ph_load(1))
            drive(ph_ln(0, 1))
            co(ph_qkv(0), ph_ln(1, 1), na=4, nb=1)
            co(attn_tail(0), ph_qkv(1), na=1, nb=1)
            co(mlp(0), attn_tail(1), na=2, nb=1)
            drive(mlp(1))
        else:
            for b in range(nbatch):
                drive(ph_load(b))
                drive(ph_ln(b, 1))
                drive(ph_qkv(b))
                drive(attn_tail(b))
                drive(mlp(b))

    nc.compile()
    return nc


def _q8(a, scale):
    return np.clip(np.asarray(a, np.float32) * scale, -240.0, 240.0).astype(
        ml_dtypes.float8_e4m3)


def _prep_inputs(x, qkv_w, qkv_b, proj_w, proj_b, n1_g, n1_b, n2_g, n2_b,
                 fc1_w, fc1_b, fc2_w, fc2_b):
    x = np.ascontiguousarray(np.asarray(x, np.float32))
    qkv_w = np.asarray(qkv_w, np.float32)
    qkv_b = np.asarray(qkv_b, np.float32)
    n1_g = np.asarray(n1_g, np.float32)
    n1_b = np.asarray(n1_b, np.float32)
    fc1_w = np.asarray(fc1_w, np.float32)
    wqk = _q8(n1_g[:, None] * qkv_w[:, :2 * C], WS)
    qkb = qkv_b[:2 * C] + n1_b @ qkv_w[:, :2 * C]
    wv = n1_g[:, None] * qkv_w[:, 2 * C:]
    vb = qkv_b[2 * C:] + n1_b @ qkv_w[:, 2 * C:]
    if np.any(vb != 0):
        raise NotImplementedError("nonzero v-bias not supported")
    wvt = _q8(np.ascontiguousarray(wv.T), WSV)
    wf1 = (np.asarray(n2_g, np.float32)[:, None] * fc1_w).astype(
        ml_dtypes.bfloat16)
    f1b = np.asarray(fc1_b, np.float32) + np.asarray(n2_b, np.float32) @ fc1_w
    pjb = np.asarray(proj_b, np.float32)
    f2b = np.asarray(fc2_b, np.float32)
    apply_pjb = bool(np.any(pjb != 0))
    apply_qkb = bool(np.any(qkb != 0))
    apply_f2b = bool(np.any(f2b != 0))
    common = {
        "wqk": wqk, "qkb": qkb.astype(np.float32), "wvt": wvt,
        "wpr": _q8(proj_w, WS), "pjb": pjb,
        "wf1": wf1, "f1b": f1b.astype(np.float32),
        "wf2": _q8(fc2_w, WS), "f2b": f2b,
    }
    in_maps = []
    for c in range(NCORES):
        m = dict(common)
        m["x"] = x[c * BL:(c + 1) * BL]
        in_maps.append(m)
    return in_maps, apply_pjb, apply_qkb, apply_f2b


_CACHE = {}


def run(inputs: dict, trace: bool = False):
    in_maps, apply_pjb, apply_qkb, apply_f2b = _prep_inputs(**inputs)
    key = (apply_pjb, apply_qkb, apply_f2b)
    if key not in _CACHE:
        _CACHE[key] = build_program(apply_pjb, apply_qkb, apply_f2b)
    nc = _CACHE[key]
    res = run_bass_kernel_spmd(nc, in_maps, core_ids=list(range(NCORES)),
                               trace=trace)
    y = np.concatenate([res.results[c]["y"] for c in range(NCORES)], axis=0)
    return y.astype(np.float32), res


def kernel(**inputs) -> np.ndarray:
    y, _ = run(inputs, trace=False)
    return y
